# revision 31
# baseline (speedup 1.0000x reference)
"""Causal self-attention (B=4, T=2048, C=1024, 16 heads) on 8 TRN2 NeuronCores.

Sharding: data-parallel over batch (4) x tensor-parallel over heads (2 groups
of 8). Each core computes qkv + attention for its 8 heads and a partial
output projection (row-parallel); the host sums the two partials per batch.

Everything on-chip runs in a transposed layout so no tensor ever needs an
on-device transpose:
  QT/KT [ch, t]  <- W^T @ x^T      (x^T prepared on host)
  attT  [k, q]   = exp(K @ Q^T / 8) * causal_mask
  yT    [ch, q]  = V_aug^T @ attT  (V augmented with a ones column per head ->
                                    row 64 of each head's block = softmax denom)
  out   [q, c]   = yT^T @ Wp       (partial; host-reduced across head groups)

All matmul operands are float16 (same 1 cyc/row PE rate as f32r, but 4x
cheaper LDWEIGHTS so weight loads hide entirely under the previous matmul,
and half the DMA/SBUF traffic). PSUM accumulation stays f32; the softmax
reciprocal is computed in f32 on the vector engine and broadcast across
partitions on the (otherwise idle) GPSIMD engine.

Schedule: per q-strip s, the B(s) attention units are emitted at per-k-tile
granularity (QK pair -> exp -> AV pair with lag 1) and woven by estimated
PE cost with independent matmul work -- A(s+1) qkv projections for strips
0-2, and ALL deferred output projections C(0..2) during B(3), where the
scalar engine's exp stream is the local bottleneck. This keeps the PE queue
from ever stalling on the exp chain (which would drop the HAM clock gate to
half rate).

Softmax skips max-subtraction (scores/8 are O(1) here, exp is safe), which is
mathematically identical to the reference; fully-masked blocks are never
computed, straddle blocks only compute the causally valid column range, and
only the diagonal 128-wide sub-block needs a mask multiply.
"""

import os
import sys

import numpy as np

for _p in ("/opt/trn_rl_repo", "/root/.axon_site/_ro/trn_rl_repo"):
    if os.path.isdir(_p) and _p not in sys.path:
        sys.path.append(_p)

import concourse.bass as bass  # noqa: E402,F401
import concourse.mybir as mybir  # noqa: E402
import concourse.tile as tile  # noqa: E402
from concourse import bacc, bass_utils  # noqa: E402

f32 = mybir.dt.float32
F16 = mybir.dt.float16

B, T, C = 4, 2048, 1024
N_HEAD, D = 16, 64
NCORES = 8
HPC = 8  # heads per core
CH = HPC * D  # 512 channels per core
P = 128
NQ = 512  # q-strip width
NSTRIP = T // NQ  # 4
SCALE = 1.0 / 8.0  # 1/sqrt(D)

MM_DT = F16
DEBUG_DUMP = False


def build():
    nc = bacc.Bacc("TRN2", target_bir_lowering=False, debug=False)
    xt = nc.dram_tensor("xt", (C, T), MM_DT, kind="ExternalInput")
    wq = nc.dram_tensor("wq", (C, CH), MM_DT, kind="ExternalInput")
    wk = nc.dram_tensor("wk", (C, CH), MM_DT, kind="ExternalInput")
    wv = nc.dram_tensor("wv", (C, CH), MM_DT, kind="ExternalInput")
    wp = nc.dram_tensor("wp", (CH, C), MM_DT, kind="ExternalInput")
    mk = nc.dram_tensor("mk", (P, 128), F16, kind="ExternalInput")
    out = nc.dram_tensor("out", (T, C), F16, kind="ExternalOutput")
    dbg = {}
    if DEBUG_DUMP:
        for m in range(4):
            dbg[f"q{m}"] = nc.dram_tensor(f"dbg_q{m}", (P, NQ), F16, kind="ExternalOutput")
            dbg[f"k{m}"] = nc.dram_tensor(f"dbg_k{m}", (P, NQ), F16, kind="ExternalOutput")
            dbg[f"v{m}"] = nc.dram_tensor(f"dbg_v{m}", (P, HPC * 65), F16, kind="ExternalOutput")
            dbg[f"y{m}"] = nc.dram_tensor(f"dbg_y{m}", (P, NQ), F16, kind="ExternalOutput")
            dbg[f"s{m}"] = nc.dram_tensor(f"dbg_s{m}", (P, NQ), f32, kind="ExternalOutput")
            dbg[f"r{m}"] = nc.dram_tensor(f"dbg_r{m}", (33, NQ), f32, kind="ExternalOutput")
    Exp = mybir.ActivationFunctionType.Exp

    with tile.TileContext(nc) as tc:
        with (
            tc.tile_pool(name="sb", bufs=1) as sb,
            tc.tile_pool(name="ps", bufs=1, space="PSUM") as psp,
        ):
            mask = sb.tile([P, 128], F16, tag="mask", bufs=1, name="mask")
            nc.sync.dma_start(mask[:], mk[:])
            col1 = sb.tile([P, HPC], F16, tag="ones8", bufs=1, name="col1")
            nc.vector.memset(col1[:], 1.0)
            # selector for the softmax-reciprocal broadcast: row 0 -> out
            # partitions 0-63 (head A), row 32 -> partitions 64-127 (head B).
            # SBUF partition bases must be 32-aligned, hence the 33-row shape.
            sel2 = sb.tile([33, 128], F16, tag="sel2", bufs=1, name="sel2")
            nc.vector.memset(sel2[:], 0.0)
            nc.vector.memset(sel2[0:1, 0:64], 1.0)
            nc.vector.memset(sel2[32:33, 64:128], 1.0)
            # pre-fill the denominator ring buffers with 1.0 so the unused
            # filler rows stay finite through the reciprocal
            for _rb in range(2):
                _dt = sb.tile([33, NQ], f32, tag="den", bufs=2, name="denz")
                nc.vector.memset(_dt[:], 1.0)


            def load_w(dram, nm, eng):
                ts_ = []
                for c in range(8):
                    t = sb.tile([P, CH], MM_DT, tag="w", bufs=32, name=f"{nm}{c}")
                    eng.dma_start(t[:], dram[c * P : (c + 1) * P, :])
                    ts_.append(t)
                return ts_

            kts = [sb.tile([P, T], MM_DT, tag="kt", bufs=4, name=f"kt{m}") for m in range(4)]
            vts = [None] * 16
            qts = {}

            # ---- Phase A: QT (strip-local), KT (transposed) and V (ones-augmented)
            def a_units(s):
                xts = []

                def u_dma():
                    for c in range(8):
                        t = sb.tile([P, NQ], MM_DT, tag="x", bufs=16, name=f"x{s}_{c}")
                        eng = nc.gpsimd if c % 2 else nc.sync
                        eng.dma_start(t[:], xt[c * P : (c + 1) * P, s * NQ : (s + 1) * NQ])
                        xts.append(t)
                    qts[s] = []

                def u_qt(m):
                    ps = psp.tile([P, NQ], f32, tag="mm", bufs=2, name="psa")
                    for c in range(8):
                        nc.tensor.matmul(
                            ps[:],
                            wq_sb[c][:, m * P : (m + 1) * P],
                            xts[c][:],
                            start=(c == 0),
                            stop=(c == 7),
                        )
                    qt_t = sb.tile([P, NQ], MM_DT, tag="qt", bufs=8, name=f"q{s}_{m}")
                    nc.vector.tensor_copy(qt_t[:], ps[:])
                    qts[s].append(qt_t)
                    if DEBUG_DUMP and s == 0:
                        nc.sync.dma_start(dbg[f"q{m}"][:], qt_t[:])

                def u_kt(m):
                    ps = psp.tile([P, NQ], f32, tag="mm", bufs=2, name="psk")
                    for c in range(8):
                        nc.tensor.matmul(
                            ps[:],
                            wk_sb[c][:, m * P : (m + 1) * P],
                            xts[c][:],
                            start=(c == 0),
                            stop=(c == 7),
                        )
                    nc.vector.tensor_copy(kts[m][:, s * NQ : (s + 1) * NQ], ps[:])
                    if DEBUG_DUMP and s == 0:
                        nc.sync.dma_start(dbg[f"k{m}"][:], kts[m][:, 0:NQ])

                def u_v(mt):
                    g = s * 4 + mt
                    ps = psp.tile([P, NQ], f32, tag="mm", bufs=2, name="psv")
                    for c in range(8):
                        nc.tensor.matmul(
                            ps[:],
                            xts[c][:, mt * P : (mt + 1) * P],
                            wv_sb[c][:],
                            start=(c == 0),
                            stop=(c == 7),
                        )
                    vt = sb.tile([P, HPC * 65], F16, tag="v", bufs=16, name=f"v{g}")
                    v3 = vt.rearrange("p (h e) -> p h e", e=65)
                    nc.vector.tensor_copy(v3[:, :, 0:64], ps.rearrange("p (h e) -> p h e", e=64))
                    nc.vector.tensor_copy(
                        v3[:, :, 64:65], col1[:].rearrange("p (h e) -> p h e", e=1)
                    )
                    vts[g] = vt
                    if DEBUG_DUMP and g < 4:
                        nc.sync.dma_start(dbg[f"v{g}"][:], vt[:])

                units = [(0, u_dma)]
                # emit one of each kind round-robin so B(s-1) never starves a class
                for m in range(4):
                    units.append((4096, lambda m=m: u_qt(m)))
                    units.append((4096, lambda m=m: u_kt(m)))
                    units.append((4096, lambda m=m: u_v(m)))
                return units

            # ---- Phase B: flash attention in transposed layout.
            # Per k-tile: QK pair (sub 0/1 = PE row groups 0-1/2-3, run
            # concurrently) -> exp on scalar -> AV pair, with the AV of
            # k-tile j emitted after the QK of k-tile j+1 (lag-1 software
            # pipeline; qkp PSUM pool is 2 deep).
            yts = {}

            def b_units(s):
                units = []
                pend_b = None
                for c in range(4):  # head pairs
                    cu, nb = bc_units(s, c)
                    if pend_b is not None:
                        cu.insert(min(5, len(cu) - 1), pend_b)
                    pend_b = nb
                    units.extend(cu)
                units.append(pend_b)  # last head pair's norm_b at strip end
                return units

            def bc_units(s, c):
                nkt = 4 * (s + 1)
                st = {}

                def get_av():
                    if "av" not in st:
                        st["av"] = [
                            psp.tile([65, NQ], f32, tag="sm", bufs=2, name=f"av{s}{c}{u}")
                            for u in range(2)
                        ]
                    return st["av"]

                def u_qk(kt):
                    j = kt - 4 * s  # >=0: diagonal-straddling tile
                    off = 128 * max(j, 0)
                    qkp = psp.tile([P, 2 * NQ], f32, tag="qk", bufs=2, name="qkp")
                    for sub in range(2):
                        nc.tensor.matmul(
                            qkp[:, sub * NQ + off : (sub + 1) * NQ],
                            kts[c][sub * 64 : (sub + 1) * 64, kt * P : (kt + 1) * P],
                            qts[s][c][sub * 64 : (sub + 1) * 64, off:NQ],
                            start=True,
                            stop=True,
                        )
                    att = sb.tile([P, 2 * NQ], F16, tag="att", bufs=4, name="att")
                    nc.scalar.activation(
                        att.rearrange("p (u q) -> p u q", u=2)[:, :, off:NQ],
                        qkp.rearrange("p (u q) -> p u q", u=2)[:, :, off:NQ],
                        Exp,
                        scale=SCALE,
                    )
                    if j >= 0:
                        for sub in range(2):
                            nc.vector.tensor_mul(
                                att[:, sub * NQ + off : sub * NQ + off + 128],
                                att[:, sub * NQ + off : sub * NQ + off + 128],
                                mask[:],
                            )
                    st[kt] = (att, off)

                def u_av(kt):
                    av = get_av()
                    att, off = st.pop(kt)
                    for sub in range(2):
                        h = 2 * c + sub
                        nc.tensor.matmul(
                            av[sub][:, off:NQ],
                            vts[kt][:, h * 65 : (h + 1) * 65],
                            att[:, sub * NQ + off : (sub + 1) * NQ],
                            start=(kt == 0),
                            stop=(kt == nkt - 1),
                        )

                def u_norm_a():
                    # free the av PSUM pair fast: yT staging copies on DVE,
                    # denominator-row copies on the scalar queue (parallel),
                    # then 1/denom + f16 cast on DVE. No PE instruction here.
                    av = st["av"]
                    stg = sb.tile([P, NQ], f32, tag="stg", bufs=2, name="stg")
                    den = sb.tile([33, NQ], f32, tag="den", bufs=2, name="den")
                    for sub in range(2):
                        nc.vector.tensor_copy(
                            stg[sub * 64 : (sub + 1) * 64, :], av[sub][0:64, :]
                        )
                        nc.scalar.copy(
                            den[32 * sub : 32 * sub + 1, :], av[sub][64:65, :]
                        )
                    rc2 = sb.tile([33, NQ], f32, tag="rc", bufs=2, name="rc")
                    nc.vector.reciprocal_approx_fast(out=rc2[:], in_=den[:])
                    rc16 = sb.tile([33, NQ], F16, tag="rc16", bufs=2, name="rc16")
                    nc.vector.tensor_copy(rc16[:], rc2[:])
                    st["norm"] = (stg, rc16, rc2)

                def u_norm_b():
                    # broadcast 1/denom to all partitions (one selector
                    # matmul) and scale yT. Emitted well after u_norm_a so
                    # the PE never waits on the DVE chain.
                    stg, rc16, rc2 = st["norm"]
                    bc_ps = psp.tile([P, NQ], f32, tag="mm", bufs=2, name="bcp")
                    nc.tensor.matmul(bc_ps[:], sel2[:], rc16[:], start=True, stop=True)
                    yts[(c, s)] = sb.tile([P, NQ], MM_DT, tag="y", bufs=16, name=f"y{c}{s}")
                    nc.vector.tensor_mul(yts[(c, s)][:], stg[:], bc_ps[:])
                    if DEBUG_DUMP and s == 0:
                        nc.sync.dma_start(dbg[f"y{c}"][:], yts[(c, s)][:])
                        nc.sync.dma_start(dbg[f"s{c}"][:], stg[:])
                        nc.sync.dma_start(dbg[f"r{c}"][:], rc2[:])

                units = [(512, lambda: u_qk(0))]
                if nkt > 1:
                    units.append((512 - 128 * max(1 - 4 * s, 0), lambda: u_qk(1)))
                for kt in range(2, nkt):
                    off = 128 * max(kt - 4 * s, 0)
                    units.append(
                        (2 * (512 - 128 * max(kt - 2 - 4 * s, 0)), lambda kt=kt: u_av(kt - 2))
                    )
                    units.append((512 - off, lambda kt=kt: u_qk(kt)))
                units.append((2 * 256, lambda: u_av(nkt - 2)))
                units.append((2 * 128, lambda: u_av(nkt - 1)))
                units.append((6000, u_norm_a))
                return units, (1200, u_norm_b)

            # ---- Phase C: partial projection (host reduces across head groups)
            def c_units(s):
                def u_proj(o, n):
                    m = 4 * s + o
                    ps = psp.tile([P, NQ], f32, tag="mm", bufs=2, name="psc")
                    for c in range(4):
                        nc.tensor.matmul(
                            ps[:],
                            yts[(c, s)][:, o * P : (o + 1) * P],
                            wp_sb[c * 2 + n][:],
                            start=(c == 0),
                            stop=(c == 3),
                        )
                    ot = sb.tile([P, NQ], F16, tag="ot", bufs=3, name="ot")
                    nc.vector.tensor_copy(ot[:], ps[:])
                    nc.sync.dma_start(out[m * P : (m + 1) * P, n * NQ : (n + 1) * NQ], ot[:])

                return [(2048, lambda o=o, n=n: u_proj(o, n)) for o in range(4) for n in range(2)]

            # ---- Driver: cost-weighted weave of primary (attention) units
            # with independent matmul work so the PE queue never stalls.
            def weave(primary, secondary):
                pt = sum(cu[0] for cu in primary) or 1
                stt = sum(cu[0] for cu in secondary)
                acc_p = 0
                acc_s = 0
                j = 0
                for cost, fn in primary:
                    fn()
                    acc_p += cost
                    while j < len(secondary) and acc_s * pt < acc_p * stt:
                        secondary[j][1]()
                        acc_s += secondary[j][0]
                        j += 1
                while j < len(secondary):
                    secondary[j][1]()
                    j += 1

            a0 = a_units(0)
            a0[0][1]()  # x(0) DMAs issue before the weight blocks
            wq_sb = load_w(wq, "wq", nc.sync)
            wk_sb = load_w(wk, "wk", nc.scalar)
            wv_sb = load_w(wv, "wv", nc.gpsimd)
            wp_sb = []
            for c in range(4):
                for n in range(2):
                    t = sb.tile([P, NQ], MM_DT, tag="w", bufs=32, name=f"wp{c}{n}")
                    nc.gpsimd.dma_start(t[:], wp[c * P : (c + 1) * P, n * NQ : (n + 1) * NQ])
                    wp_sb.append(t)

            for _, u in a0[1:]:
                u()
            weave(b_units(0), a_units(1))
            weave(b_units(1), a_units(2))
            weave(b_units(2), a_units(3))
            weave(b_units(3), c_units(0) + c_units(1))
            for _, u in c_units(2) + c_units(3):
                u()

    nc.compile()
    return nc


_NC = None


def _get_nc():
    global _NC
    if _NC is None:
        _NC = build()
    return _NC


def host_mask():
    # diagonal-block causal mask: keep k <= q within a 128x128 block
    m = np.zeros((P, P), np.float16)
    for kk in range(P):
        m[kk, kk:] = 1.0
    return m


def make_in_maps(x, w_qkv, w_proj):
    x = np.asarray(x, np.float32)
    w_qkv = np.asarray(w_qkv, np.float16)
    w_proj = np.asarray(w_proj, np.float16)
    mkm = host_mask()
    xts = [np.ascontiguousarray(x[b].T).astype(np.float16) for b in range(B)]
    in_maps = []
    for core in range(NCORES):
        b, hg = core // 2, core % 2
        lo, hi = hg * CH, (hg + 1) * CH
        in_maps.append(
            {
                "xt": xts[b],
                "wq": np.ascontiguousarray(w_qkv[:, lo:hi]),
                "wk": np.ascontiguousarray(w_qkv[:, C + lo : C + hi]),
                "wv": np.ascontiguousarray(w_qkv[:, 2 * C + lo : 2 * C + hi]),
                "wp": np.ascontiguousarray(w_proj[lo:hi, :]),
                "mk": mkm,
            }
        )
    return in_maps


def kernel(x, w_qkv, w_proj):
    in_maps = make_in_maps(x, w_qkv, w_proj)
    last_err = None
    for attempt in range(3):
        try:
            res = bass_utils.run_bass_kernel_spmd(
                _get_nc(), in_maps, core_ids=list(range(NCORES))
            )
            break
        except Exception as e:  # transient device wedge: back off and retry
            last_err = e
            import time

            time.sleep(10 * (attempt + 1))
    else:
        raise last_err
    out = np.empty((B, T, C), np.float32)
    for b in range(B):
        out[b] = res.results[2 * b]["out"].astype(np.float32) + res.results[
            2 * b + 1
        ]["out"].astype(np.float32)
    return out
